# revision 11
# baseline (speedup 1.0000x reference)
"""Trainium2 Bass kernel for nn_ByteEncoder.

Model (see harness reference): byte + 6 n-gram hash embeddings summed -> one
post-norm transformer encoder layer (MHA + relu FFN) -> cross-attention from
patch-boundary queries to the full sequence.

Sharding: 8 cores; core c handles batch b=c//2, sequence half h=c%2
(1024 tokens).  The ~1.2GB embedding tables are replicated per core (bf16,
pre-scaled by 1/7) and gathered on-device with one fused indirect DMA per
128-token tile.  Self-attn K/V is exchanged pair-wise with a bf16 AllGather
that overlaps the local-key half of the attention sweep; the remote half is
pulled from the gathered buffer with a per-core indirect row gather (so the
same SPMD program works for both pair parities).  Scores/probs, the FFN, and
the whole cross-attention path run in bf16; exp is batched over two PSUM
banks per activation call.
"""

import sys
import numpy as np

sys.path.insert(0, "/opt/trn_rl_repo")

import concourse.bass as bass
import concourse.bacc as bacc
import concourse.tile as tile
import concourse.mybir as mybir
from concourse.bass_utils import run_bass_kernel_spmd
from concourse.masks import make_identity

F32 = mybir.dt.float32
F32R = mybir.dt.float32r
BF16 = mybir.dt.bfloat16
I32 = mybir.dt.int32
AF = mybir.ActivationFunctionType

B, S, D, H, V, P = 4, 2048, 512, 8, 100000, 256
NGRAMS = list(range(3, 9))
NT = 1 + len(NGRAMS)          # 7 tables (byte + 6 ngram)
DH = D // H                   # 64
DF = 4 * D                    # 2048
SCALE = float(np.float32(DH) ** -0.5)
N_CORES = 8
SL = S // 2                   # 1024 local tokens
PL = P // 2                   # 128 local queries
KT = D // 128                 # 4 k-tiles over D
TT_L = SL // 128              # 8 local token tiles
TT_F = S // 128               # 16 full token tiles
FT = DF // 128                # 16 tiles over d_ff
VROWS = 256 + len(NGRAMS) * V # combined table rows

# bf16 elements per partition in the kv exchange row: K^T block then V' block
KV_K = KT * SL                # 4096
KV_V = TT_L * H * (DH + 1)    # 4160
KV_ROW = KV_K + KV_V          # 8256

# bf16-element offsets inside the x2 bounce buffer
X2T_ELE = D * SL                       # X2^T block
X2_ELE = SL * D                        # token-major x2 block
X2B_ELE = X2T_ELE + X2_ELE

_WF32 = ["sWq", "sWk", "sWv", "sWo"]
_WBF = ["cWq", "cWk", "cWv", "cWo", "W1", "W2"]
_BVEC = ["sbq", "sbk", "sbv", "sbo", "b2", "cbq", "cbk", "cbv", "cbo",
         "ln1g", "ln1b", "ln2g", "ln2b"]


def _build_program(stage="H", vrows=VROWS, dbg=False):
    nc = bacc.Bacc("TRN2", target_bir_lowering=False, debug=False,
                   num_devices=N_CORES)
    dt = {}
    if dbg:
        for nm in ["dbg_emb", "dbg_ktl", "dbg_ktr", "dbg_otok", "dbg_x1",
                   "dbg_x2", "dbg_qg"]:
            dt[nm] = nc.dram_tensor(nm, [128, D], F32, kind="ExternalOutput").ap()
    dt["table"] = nc.dram_tensor("table", [vrows, D], BF16, kind="ExternalInput").ap()
    dt["idx"] = nc.dram_tensor("idx", [128, TT_L, NT], I32, kind="ExternalInput").ap()
    dt["qidx"] = nc.dram_tensor("qidx", [128, 2], I32, kind="ExternalInput").ap()
    dt["qmask"] = nc.dram_tensor("qmask", [128, 2], F32, kind="ExternalInput").ap()
    dt["qsel"] = nc.dram_tensor("qsel", [128, 1], I32, kind="ExternalInput").ap()
    dt["rrow"] = nc.dram_tensor("rrow", [128, 1], I32, kind="ExternalInput").ap()
    for w in _WF32:
        dt[w] = nc.dram_tensor(w, [D, D], F32, kind="ExternalInput").ap()
    for w in ["cWq", "cWk", "cWv", "cWo"]:
        dt[w] = nc.dram_tensor(w, [D, D], BF16, kind="ExternalInput").ap()
    dt["W1"] = nc.dram_tensor("W1", [D, DF], BF16, kind="ExternalInput").ap()
    dt["W2"] = nc.dram_tensor("W2", [DF, D], BF16, kind="ExternalInput").ap()
    dt["b1"] = nc.dram_tensor("b1", [DF], F32, kind="ExternalInput").ap()
    for bv in _BVEC:
        dt[bv] = nc.dram_tensor(bv, [D], F32, kind="ExternalInput").ap()
    out_d = nc.dram_tensor("out", [PL, D], F32, kind="ExternalOutput").ap()

    # DRAM bounce buffers for the pair collectives (all bf16)
    kv_in = nc.dram_tensor("kv_in", [128, KV_ROW], BF16, kind="Internal").ap()
    kv_all = nc.dram_tensor("kv_all", [2 * 128, KV_ROW], BF16, kind="Internal").ap()
    x2loc = nc.dram_tensor("x2loc", [SL, D], BF16, kind="Internal").ap()
    qr_in = nc.dram_tensor("qr_in", [P, D], BF16, kind="Internal").ap()
    qr_sum = nc.dram_tensor("qr_sum", [P, D], BF16, kind="Internal").ap()
    av_in = nc.dram_tensor("av_in", [H, DH + 1, P], F32, kind="Internal").ap()
    av_sum = nc.dram_tensor("av_sum", [H, DH + 1, P], F32, kind="Internal").ap()
    obuf = nc.dram_tensor("obuf", [P, D], F32, kind="Internal").ap()
    cc = dict(kv_in=kv_in, kv_all=kv_all, x2loc=x2loc, qr_in=qr_in,
              qr_sum=qr_sum, av_in=av_in, av_sum=av_sum, obuf=obuf)
    groups = [[0, 1], [2, 3], [4, 5], [6, 7]]

    with tile.TileContext(nc) as tc:
        _emit(nc, tc, dt, out_d, cc, groups, stage, dbg)
    nc.compile()
    return nc


def _mm_acc(nc, ps, lhsT_tiles, rhs_tiles):
    n = len(lhsT_tiles)
    for k in range(n):
        nc.tensor.matmul(ps, lhsT=lhsT_tiles[k], rhs=rhs_tiles[k],
                         start=(k == 0), stop=(k == n - 1))


def _emit(nc, tc, dt, out_d, cc, groups, stage="H", dbg=False):
    kv_in, kv_all = cc["kv_in"], cc["kv_all"]
    x2loc, qr_in, qr_sum = cc["x2loc"], cc["qr_in"], cc["qr_sum"]
    av_in, av_sum, obuf = cc["av_in"], cc["av_sum"], cc["obuf"]
    from contextlib import ExitStack

    ctx = ExitStack()
    with ctx:
        # One big pool; tensors with disjoint lifetimes share a slot via the
        # same tag (bufs=1 -> strict sequential reuse, enforced by tile deps).
        big = ctx.enter_context(tc.tile_pool(name="big", bufs=1))
        pers = ctx.enter_context(tc.tile_pool(name="pers", bufs=1))
        pExp = ctx.enter_context(tc.tile_pool(name="pExp", bufs=2))
        psT = ctx.enter_context(tc.tile_pool(name="psT", bufs=2, space="PSUM"))
        ps = ctx.enter_context(tc.tile_pool(name="ps", bufs=2, space="PSUM"))
        psAV = ctx.enter_context(tc.tile_pool(name="psAV", bufs=1, space="PSUM"))

        identF = pers.tile([128, 128], F32)
        make_identity(nc, identF[:])
        identB = pers.tile([128, 128], BF16)
        make_identity(nc, identB[:])
        epsT = pers.tile([128, 1], F32)
        nc.vector.memset(epsT[:], 1e-5)
        ones64 = pers.tile([128, TT_F * H], F32)
        nc.vector.memset(ones64[:], 1.0)

        # broadcast-along-free bias rows, chained 4-row slots
        def load_bcast(tile_, i, name):
            src = dt[name]
            bc_ap = bass.AP(tensor=src.tensor, offset=src.offset,
                            ap=[[0, 128]] + list(src.ap))
            nc.gpsimd.dma_start(out=tile_[:, i, :], in_=bc_ap)
            return tile_[:, i, :]

        bc1 = big.tile([128, 4, D], F32, tag="bc")
        bcast = {}
        for i, name in enumerate(["sbv", "sbo", "ln1g", "ln1b"]):
            bcast[name] = load_bcast(bc1, i, name)
        # per-partition (feature-major) bias tiles
        pp = {}
        for name in ["sbq", "sbk", "cbq", "cbk", "b2"]:
            t = pers.tile([128, KT], F32, tag=f"pp_{name}")
            nc.sync.dma_start(out=t[:], in_=dt[name].rearrange("(dp p) -> p dp", p=128))
            pp[name] = t
        b1_s = pers.tile([128, FT], F32)
        nc.sync.dma_start(out=b1_s[:], in_=dt["b1"].rearrange("(dp p) -> p dp", p=128))

        # self-attn QKV weights: one 24KB tile
        sWqkv = big.tile([128, 3, KT, D], F32R, tag="s1")
        for i, name in enumerate(["sWq", "sWk", "sWv"]):
            nc.sync.dma_start(
                out=sWqkv[:, i, :, :],
                in_=dt[name].bitcast(F32R).rearrange("(kt p) n -> p kt n", p=128))
        sWq_s, sWk_s, sWv_s = sWqkv[:, 0], sWqkv[:, 1], sWqkv[:, 2]

        # ---------------- Phase A: gather + embeds + X^T ----------------
        idx_t = pers.tile([128, TT_L, NT], I32)
        nc.sync.dma_start(idx_t[:], dt["idx"][:])
        emb = big.tile([128, TT_L, D], F32, tag="emb")
        for tt in range(TT_L):
            # HW indirect DMA gathers one row per partition per call
            emb7 = big.tile([128, NT, D], BF16, tag=("e7a" if tt % 2 else "e7b"))
            for j in range(NT):
                nc.gpsimd.indirect_dma_start(
                    out=emb7[:, j, :], out_offset=None, in_=dt["table"][:],
                    in_offset=bass.IndirectOffsetOnAxis(ap=idx_t[:, tt, j:j + 1], axis=0))
            nc.vector.tensor_add(emb[:, tt, :], emb7[:, 0, :], emb7[:, 1, :])
            for j in range(2, NT):
                nc.vector.tensor_add(emb[:, tt, :], emb[:, tt, :], emb7[:, j, :])

        def _dump(name, ap_f32=None, ap_low=None):
            if not dbg:
                return
            if ap_low is not None:
                tl = pers.tile([128, D], F32, tag="dbgt")
                nc.vector.tensor_copy(tl[:].rearrange("p (a b) -> p a b", a=ap_low.shape[1]) if len(ap_low.shape) == 3 else tl[:], ap_low)
                nc.sync.dma_start(dt[name][:], tl[:])
            else:
                nc.sync.dma_start(dt[name][:], ap_f32)

        _dump("dbg_emb", ap_f32=emb[:, 0, :])
        if stage == "A":
            nc.sync.dma_start(out_d[:], emb[:, 0, :])
            return
        XT = big.tile([128, KT, SL], F32R, tag="sXT")
        for tt in range(TT_L):
            for dp in range(KT):
                pt = psT.tile([128, 128], F32, tag="pt")
                nc.tensor.transpose(pt[:], emb[:, tt, dp * 128:(dp + 1) * 128], identF[:])
                nc.vector.tensor_copy(XT[:, dp, tt * 128:(tt + 1) * 128], pt[:])

        # ---------------- Phase B: QKV projections (local tokens) ----------------
        QT = big.tile([128, KT, SL], BF16, tag="e7a")
        KTl = big.tile([128, KT, SL], BF16, tag="e7b")
        V1l = big.tile([128, TT_L, H, DH + 1], BF16, tag="v1l")
        nc.vector.tensor_copy(
            V1l[:, :, :, DH:DH + 1].rearrange("p a b c -> p (a b c)"),
            ones64[:, 0:TT_L * H])
        for dst, w_s, b_s in ((QT, sWq_s, pp["sbq"]), (KTl, sWk_s, pp["sbk"])):
            for dp in range(KT):
                for c2 in range(SL // 512):
                    ps_ = ps.tile([128, 512], F32, tag="ps")
                    _mm_acc(nc, ps_[:],
                            [w_s[:, k, dp * 128:(dp + 1) * 128] for k in range(KT)],
                            [XT[:, k, c2 * 512:(c2 + 1) * 512] for k in range(KT)])
                    nc.scalar.activation(dst[:, dp, c2 * 512:(c2 + 1) * 512], ps_[:],
                                         AF.Identity, bias=b_s[:, dp:dp + 1])
        for tt in range(TT_L):
            ps_ = ps.tile([128, 512], F32, tag="ps")
            _mm_acc(nc, ps_[:],
                    [XT[:, k, tt * 128:(tt + 1) * 128] for k in range(KT)],
                    [sWv_s[:, k, :] for k in range(KT)])
            nc.vector.tensor_add(
                V1l[:, tt, :, 0:DH],
                ps_[:].rearrange("p (h d) -> p h d", h=H),
                bcast["sbv"].rearrange("p (h d) -> p h d", h=H))

        # ---------------- Phase C: AllGather K^T and V' (bf16) ----------------
        nc.sync.dma_start(
            out=kv_in[:, 0:KV_K].rearrange("p (dp t) -> p dp t", t=SL),
            in_=KTl[:])
        nc.sync.dma_start(
            out=kv_in[:, KV_K:KV_ROW].rearrange("p (a b c) -> p a b c", a=TT_L, b=H),
            in_=V1l[:])
        nc.gpsimd.collective_compute(
            "AllGather", mybir.AluOpType.bypass, replica_groups=groups,
            ins=[kv_in.opt()], outs=[kv_all.opt()])
        # pull only the partner's rows (per-core row indices solve parity)
        rrow_t = pers.tile([128, 1], I32)
        nc.sync.dma_start(rrow_t[:], dt["rrow"][:])
        kvr = big.tile([128, KV_ROW], BF16, tag="s2")
        nc.gpsimd.indirect_dma_start(
            out=kvr[:], out_offset=None, in_=kv_all[:],
            in_offset=bass.IndirectOffsetOnAxis(ap=rrow_t[:, 0:1], axis=0))
        KTr = kvr[:, 0:KV_K].rearrange("p (dp t) -> p dp t", t=SL)
        V1r = kvr[:, KV_K:KV_ROW].rearrange("p (a b c) -> p a b c", a=TT_L, b=H)
        _dump("dbg_ktl", ap_low=KTl[:, :, 0:128])
        _dump("dbg_ktr", ap_low=KTr[:, :, 0:128])

        if stage == "C":
            tmpc = pers.tile([128, D], F32, tag="tmpc")
            nc.vector.tensor_copy(tmpc[:].rearrange("p (a b) -> p a b", a=KT),
                                  KTr[:, :, 0:128])
            nc.sync.dma_start(out_d[:], tmpc[:])
            return
        # ---------------- Phase D: self-attention (local half first) ----------
        o_acc = big.tile([128, H, TT_L, DH + 1], F32, tag="s3")
        O_tok = big.tile([128, TT_L, D], F32R, tag="otok")
        for half in range(2):
            Ksrc = KTl if half == 0 else KTr
            Vsrc = V1l if half == 0 else V1r
            for h in range(H):
                hp, hr = h // 2, (h % 2) * DH
                av = psAV.tile([128, TT_L, 128], F32, tag="av")
                for tkt in range(TT_L):
                    psS = ps.tile([128, 2, 512], F32, tag="ps")
                    for c2 in range(2):
                        nc.tensor.matmul(
                            psS[:, c2, :],
                            lhsT=Ksrc[hr:hr + DH, hp, tkt * 128:(tkt + 1) * 128],
                            rhs=QT[hr:hr + DH, hp, c2 * 512:(c2 + 1) * 512],
                            start=True, stop=True)
                    expT = pExp.tile([128, SL], BF16, tag="expT")
                    nc.scalar.activation(
                        expT[:], psS[:].rearrange("p a b -> p (a b)"),
                        AF.Exp, scale=SCALE)
                    for tqt in range(TT_L):
                        nc.tensor.matmul(
                            av[:, tqt, 0:DH + 1],
                            lhsT=expT[:, tqt * 128:(tqt + 1) * 128],
                            rhs=Vsrc[:, tkt, h, :],
                            start=(tkt == 0), stop=(tkt == TT_L - 1))
                if half == 0:
                    nc.vector.tensor_copy(o_acc[:, h, :, :], av[:, :, 0:DH + 1])
                else:
                    nc.vector.tensor_add(o_acc[:, h, :, :], o_acc[:, h, :, :],
                                         av[:, :, 0:DH + 1])
                    for tqt in range(TT_L):
                        rcp = pers.tile([128, 1], F32, tag="rcp")
                        nc.vector.reciprocal(rcp[:], o_acc[:, h, tqt, DH:DH + 1])
                        nc.vector.tensor_scalar_mul(
                            O_tok[:, tqt, h * DH:(h + 1) * DH],
                            in0=o_acc[:, h, tqt, 0:DH], scalar1=rcp[:])

        _dump("dbg_otok", ap_f32=O_tok[:, 0, :].bitcast(F32))
        if stage == "D":
            nc.sync.dma_start(out_d[:], O_tok[:, 0, :].bitcast(F32))
            return
        # ---------------- Phase E: O^T, O-proj, +emb, LN1 ----------------
        sWo_s = big.tile([128, KT, D], F32R, tag="e7a")
        nc.sync.dma_start(
            out=sWo_s[:], in_=dt["sWo"].bitcast(F32R).rearrange("(kt p) n -> p kt n", p=128))
        OT = big.tile([128, KT, SL], F32R, tag="sXT")
        for tt in range(TT_L):
            for dp in range(KT):
                pt = psT.tile([128, 128], F32, tag="pt")
                nc.tensor.transpose(pt[:], O_tok[:, tt, dp * 128:(dp + 1) * 128].bitcast(F32), identF[:])
                nc.vector.tensor_copy(OT[:, dp, tt * 128:(tt + 1) * 128], pt[:].bitcast(F32R))
        x1 = big.tile([128, TT_L, D], F32, tag="x1")
        for tt in range(TT_L):
            ps_ = ps.tile([128, 512], F32, tag="ps")
            _mm_acc(nc, ps_[:],
                    [OT[:, k, tt * 128:(tt + 1) * 128] for k in range(KT)],
                    [sWo_s[:, k, :] for k in range(KT)])
            t0 = pers.tile([128, D], F32, tag="lnt0")
            nc.vector.tensor_add(t0[:], ps_[:], bcast["sbo"])
            nc.vector.tensor_add(t0[:], t0[:], emb[:, tt, :])
            _layernorm(nc, pers, x1[:, tt, :], t0[:], bcast["ln1g"], bcast["ln1b"], epsT)
        X1T = big.tile([128, KT, SL], BF16, tag="e7b")
        for tt in range(TT_L):
            for dp in range(KT):
                pt = psT.tile([128, 128], F32, tag="pt")
                nc.tensor.transpose(pt[:], x1[:, tt, dp * 128:(dp + 1) * 128], identF[:])
                nc.vector.tensor_copy(X1T[:, dp, tt * 128:(tt + 1) * 128], pt[:])

        _dump("dbg_x1", ap_f32=x1[:, 0, :])
        if stage == "E":
            nc.sync.dma_start(out_d[:], x1[:, 0, :])
            return
        # ---------------- Phase F: FFN (bf16) + LN2 -> x2, X2T ----------------
        bc2 = big.tile([128, 4, D], F32, tag="bc")
        for i, name in enumerate(["ln2g", "ln2b", "cbv", "cbo"]):
            bcast[name] = load_bcast(bc2, i, name)
        W1_s = big.tile([128, KT, DF], BF16, tag="s1")
        nc.sync.dma_start(
            out=W1_s[:], in_=dt["W1"].rearrange("(kt p) n -> p kt n", p=128))
        W2_s = big.tile([128, FT, D], BF16, tag="s2")
        nc.sync.dma_start(
            out=W2_s[:], in_=dt["W2"].rearrange("(kt p) n -> p kt n", p=128))
        x2 = big.tile([128, TT_L, D], BF16, tag="otok")
        X2T = big.tile([128, KT, SL], BF16, tag="sXT")
        for c2 in range(SL // 512):
            HT = big.tile([128, FT, 512], BF16, tag="s3")
            for ft in range(FT):
                ps_ = ps.tile([128, 512], F32, tag="ps")
                _mm_acc(nc, ps_[:],
                        [W1_s[:, k, ft * 128:(ft + 1) * 128] for k in range(KT)],
                        [X1T[:, k, c2 * 512:(c2 + 1) * 512] for k in range(KT)])
                nc.scalar.activation(HT[:, ft, :], ps_[:], AF.Relu,
                                     bias=b1_s[:, ft:ft + 1])
            for dp in range(KT):
                ps_ = ps.tile([128, 512], F32, tag="ps")
                _mm_acc(nc, ps_[:],
                        [W2_s[:, k, dp * 128:(dp + 1) * 128] for k in range(FT)],
                        [HT[:, k, :] for k in range(FT)])
                fft = pers.tile([128, 512], F32, tag="fft")
                nc.scalar.activation(fft[:], ps_[:], AF.Identity, bias=pp["b2"][:, dp:dp + 1])
                for st in range(4):
                    tt = c2 * 4 + st
                    pt = psT.tile([128, 128], F32, tag="pt")
                    nc.tensor.transpose(pt[:], fft[:, st * 128:(st + 1) * 128], identF[:])
                    nc.vector.tensor_add(x2[:, tt, dp * 128:(dp + 1) * 128], pt[:],
                                         x1[:, tt, dp * 128:(dp + 1) * 128])
        for tt in range(TT_L):
            _layernorm(nc, pers, x2[:, tt, :], x2[:, tt, :], bcast["ln2g"],
                       bcast["ln2b"], epsT)
            for dp in range(KT):
                pt = psT.tile([128, 128], BF16, tag="pt")
                nc.tensor.transpose(pt[:], x2[:, tt, dp * 128:(dp + 1) * 128], identB[:])
                nc.vector.tensor_copy(X2T[:, dp, tt * 128:(tt + 1) * 128], pt[:])

        _dump("dbg_x2", ap_low=x2[:, 0, :])
        if stage == "F":
            tmpf = pers.tile([128, D], F32, tag="tmpc")
            nc.vector.tensor_copy(tmpf[:], x2[:, 0, :])
            nc.sync.dma_start(out_d[:], tmpf[:])
            return
        # ------- Phase G': local x2 rows + query-row pair exchange -------
        nc.sync.dma_start(
            out=x2loc.rearrange("(tt p) d -> p tt d", p=128),
            in_=x2[:])
        qidx_t = pers.tile([128, 2], I32)
        nc.sync.dma_start(qidx_t[:], dt["qidx"][:])
        qmask_t = pers.tile([128, 2], F32)
        nc.sync.dma_start(qmask_t[:], dt["qmask"][:])
        qsel_t = pers.tile([128, 1], I32)
        nc.sync.dma_start(qsel_t[:], dt["qsel"][:])
        qga = pers.tile([128, 2, D], BF16, tag="qga")
        for qb in range(2):
            nc.gpsimd.indirect_dma_start(
                out=qga[:, qb, :], out_offset=None, in_=x2loc[:],
                in_offset=bass.IndirectOffsetOnAxis(ap=qidx_t[:, qb:qb + 1], axis=0))
            nc.vector.tensor_scalar_mul(qga[:, qb, :], in0=qga[:, qb, :],
                                        scalar1=qmask_t[:, qb:qb + 1])
        nc.sync.dma_start(
            out=qr_in.rearrange("(qb p) d -> p qb d", p=128), in_=qga[:])
        nc.gpsimd.collective_compute(
            "AllReduce", mybir.AluOpType.add, replica_groups=groups,
            ins=[qr_in.opt()], outs=[qr_sum.opt()])
        # cross K/V from the local sequence half (overlaps the AllReduce)
        cWall = big.tile([128, 4, KT, D], BF16, tag="s2")
        for i, name in enumerate(["cWq", "cWk", "cWv", "cWo"]):
            nc.sync.dma_start(
                out=cWall[:, i, :, :],
                in_=dt[name].rearrange("(kt p) n -> p kt n", p=128))
        cWq_s, cWk_s, cWv_s, cWo_s = (cWall[:, i] for i in range(4))
        cKTl = big.tile([128, KT, SL], BF16, tag="e7b")
        for dp in range(KT):
            for c2 in range(SL // 512):
                ps_ = ps.tile([128, 512], F32, tag="ps")
                _mm_acc(nc, ps_[:],
                        [cWk_s[:, k, dp * 128:(dp + 1) * 128] for k in range(KT)],
                        [X2T[:, k, c2 * 512:(c2 + 1) * 512] for k in range(KT)])
                nc.scalar.activation(cKTl[:, dp, c2 * 512:(c2 + 1) * 512], ps_[:],
                                     AF.Identity, bias=pp["cbk"][:, dp:dp + 1])
        cV1l = big.tile([128, TT_L, H, DH + 1], BF16, tag="v1l")
        nc.vector.tensor_copy(
            cV1l[:, :, :, DH:DH + 1].rearrange("p a b c -> p (a b c)"),
            ones64[:, 0:TT_L * H])
        for tt in range(TT_L):
            ps_ = ps.tile([128, 512], F32, tag="ps")
            _mm_acc(nc, ps_[:],
                    [X2T[:, k, tt * 128:(tt + 1) * 128] for k in range(KT)],
                    [cWv_s[:, k, :] for k in range(KT)])
            nc.vector.tensor_add(
                cV1l[:, tt, :, 0:DH],
                ps_[:].rearrange("p (h d) -> p h d", h=H),
                bcast["cbv"].rearrange("p (h d) -> p h d", h=H))
        # all 256 batch queries -> q^T -> cQ^T
        qfull = pers.tile([128, 2, D], BF16, tag="qga")
        nc.sync.dma_start(
            out=qfull[:], in_=qr_sum.rearrange("(qb p) d -> p qb d", p=128))
        _dump("dbg_qg", ap_low=qfull[:, 0, :])
        qT2 = pers.tile([128, KT, 2, 128], BF16, tag="qT")
        for qb in range(2):
            for dp in range(KT):
                pt = psT.tile([128, 128], BF16, tag="pt")
                nc.tensor.transpose(pt[:], qfull[:, qb, dp * 128:(dp + 1) * 128],
                                    identB[:])
                nc.vector.tensor_copy(qT2[:, dp, qb, :], pt[:])
        cQT = pers.tile([128, KT, P], BF16, tag="cQT")
        for dp in range(KT):
            ps_ = ps.tile([128, P], F32, tag="ps")
            _mm_acc(nc, ps_[:],
                    [cWq_s[:, k, dp * 128:(dp + 1) * 128] for k in range(KT)],
                    [qT2[:, k, :, :].rearrange("p a b -> p (a b)")
                     for k in range(KT)])
            nc.scalar.activation(cQT[:, dp, :], ps_[:], AF.Identity,
                                 bias=pp["cbq"][:, dp:dp + 1])

        # ------- Phase H': partial cross-attention over local keys -------
        avH = big.tile([128, H, P], F32, tag="e7a")
        for h in range(H):
            hp, hr = h // 2, (h % 2) * DH
            av = psAV.tile([128, P], F32, tag="av")
            for t2 in range(TT_L // 2):
                ps2 = ps.tile([128, 2, P], F32, tag="ps")
                for i in range(2):
                    tkt = t2 * 2 + i
                    nc.tensor.matmul(
                        ps2[:, i, :],
                        lhsT=cKTl[hr:hr + DH, hp, tkt * 128:(tkt + 1) * 128],
                        rhs=cQT[hr:hr + DH, hp, :], start=True, stop=True)
                ecl = pExp.tile([128, 2 * P], BF16, tag="expT")
                nc.scalar.activation(ecl[:], ps2[:].rearrange("p a b -> p (a b)"),
                                     AF.Exp, scale=SCALE)
                for i in range(2):
                    tkt = t2 * 2 + i
                    nc.tensor.matmul(
                        av[0:DH + 1, :],
                        lhsT=cV1l[:, tkt, h, :],
                        rhs=ecl[:, i * P:(i + 1) * P],
                        start=(tkt == 0), stop=(tkt == TT_L - 1))
            nc.vector.tensor_copy(avH[0:DH + 1, h, :], av[0:DH + 1, :])
        nc.sync.dma_start(
            out=av_in.rearrange("h r q -> r h q"), in_=avH[0:DH + 1, :, :])
        nc.gpsimd.collective_compute(
            "AllReduce", mybir.AluOpType.add, replica_groups=groups,
            ins=[av_in.opt()], outs=[av_sum.opt()])
        # reload: numerators feature-major; denominators partition-broadcast
        avf = pers.tile([128, KT, P], F32, tag="avf")
        for h2 in range(2):
            nc.sync.dma_start(
                out=avf[h2 * DH:(h2 + 1) * DH, :, :],
                in_=av_sum.rearrange("(hp h2) r q -> h2 r hp q", h2=2)[h2, 0:DH, :, :])
        denb = pers.tile([128, KT, P], F32, tag="denb")
        for h in range(H):
            src_ = av_sum[h, DH, :]
            bc_ap = bass.AP(tensor=src_.tensor, offset=src_.offset,
                            ap=[[0, DH]] + list(src_.ap))
            nc.gpsimd.dma_start(
                out=denb[(h % 2) * DH:(h % 2) * DH + DH, h // 2, :], in_=bc_ap)
        rden = pers.tile([128, KT, P], F32, tag="rden")
        nc.vector.reciprocal(rden[:].rearrange("p a b -> p (a b)"),
                             denb[:].rearrange("p a b -> p (a b)"))
        OcT = pers.tile([128, KT, P], BF16, tag="OcT")
        nc.vector.tensor_mul(OcT[:].rearrange("p a b -> p (a b)"),
                             avf[:].rearrange("p a b -> p (a b)"),
                             rden[:].rearrange("p a b -> p (a b)"))
        OF = pers.tile([128, 2, D], F32, tag="qga")
        for qb in range(2):
            ps_ = ps.tile([128, 512], F32, tag="ps")
            _mm_acc(nc, ps_[:],
                    [OcT[:, k, qb * 128:(qb + 1) * 128] for k in range(KT)],
                    [cWo_s[:, k, :] for k in range(KT)])
            nc.vector.tensor_add(OF[:, qb, :], ps_[:], bcast["cbo"])
        nc.sync.dma_start(out=obuf.rearrange("(qb p) d -> p qb d", p=128),
                          in_=OF[:])
        outsb = pers.tile([128, D], F32, tag="outsb")
        nc.gpsimd.indirect_dma_start(
            out=outsb[:], out_offset=None, in_=obuf[:],
            in_offset=bass.IndirectOffsetOnAxis(ap=qsel_t[:, 0:1], axis=0))
        nc.sync.dma_start(out_d[:], outsb[:])


def _layernorm(nc, pool, out_ap, in_ap, g_b, b_b, epsT):
    st = pool.tile([128, 6], F32, tag="ln_st")
    nc.vector.bn_stats(out=st[:], in_=in_ap)
    mv = pool.tile([128, 2], F32, tag="ln_mv")
    nc.vector.bn_aggr(out=mv[:], in_=st[:])
    sd = pool.tile([128, 1], F32, tag="ln_sd")
    nc.scalar.activation(sd[:], mv[:, 1:2], AF.Sqrt, bias=epsT[:])
    nc.vector.reciprocal(sd[:], sd[:])
    tmp = pool.tile([128, D], F32, tag="ln_tmp")
    nc.vector.tensor_scalar(out=tmp[:], in0=in_ap, scalar1=mv[:, 0:1], scalar2=sd[:],
                            op0=mybir.AluOpType.subtract, op1=mybir.AluOpType.mult)
    nc.vector.tensor_mul(tmp[:], tmp[:], g_b[:])
    nc.vector.tensor_add(out_ap, tmp[:], b_b[:])


def _ngram_hashes(bytes_seq):
    """int64-wraparound n-gram hashes, mod V.  [B, S] -> [len(NGRAMS), B, S]"""
    b = bytes_seq.astype(np.int64)
    out = np.zeros((len(NGRAMS), b.shape[0], S), dtype=np.int64)
    for j, n in enumerate(NGRAMS):
        h = np.zeros_like(b)
        for k in range(n):
            shift = n - 1 - k
            mult = np.int64(256) ** k  # wraps for n=8, matching torch/jax int64
            shifted = np.zeros_like(b)
            shifted[:, shift:] = b[:, : S - shift]
            h = h + shifted * mult
        h = np.where(np.arange(S)[None, :] >= (n - 1), h, 0)
        out[j] = h % V
    return out


_PROGRAM = None


def _get_program():
    global _PROGRAM
    if _PROGRAM is None:
        _PROGRAM = _build_program()
    return _PROGRAM


def make_in_maps(inputs):
    import ml_dtypes

    bytes_seq = np.asarray(inputs["bytes_seq"])
    patch_idx = np.asarray(inputs["patch_idx"])
    byte_emb = np.asarray(inputs["byte_emb"], dtype=np.float32)
    ngram_emb = np.asarray(inputs["ngram_emb"], dtype=np.float32)

    table = np.concatenate([byte_emb, ngram_emb.reshape(len(NGRAMS) * V, D)], axis=0)
    assert table.shape == (VROWS, D)
    # fold the 1/7 mean normalization into the (bf16) table
    table = (table * np.float32(1.0 / NT)).astype(ml_dtypes.bfloat16)
    hashes = _ngram_hashes(bytes_seq)

    weights = {}
    for w in _WF32 + ["b1"] + _BVEC:
        weights[w] = np.ascontiguousarray(np.asarray(inputs[w], dtype=np.float32))
    for w in _WBF:
        weights[w] = np.ascontiguousarray(
            np.asarray(inputs[w], dtype=np.float32).astype(ml_dtypes.bfloat16))

    in_maps = []
    for c in range(N_CORES):
        b, hh = c // 2, c % 2
        tok0 = hh * SL
        p_ar = np.arange(128)[:, None]          # [128, 1]
        tt_ar = np.arange(TT_L)[None, :]        # [1, TT_L]
        tok = tok0 + tt_ar * 128 + p_ar         # [128, TT_L]
        idx = np.zeros((128, TT_L, NT), dtype=np.int32)
        idx[:, :, 0] = bytes_seq[b][tok].astype(np.int32)
        for j in range(len(NGRAMS)):
            idx[:, :, 1 + j] = (256 + j * V + hashes[j, b][tok]).astype(np.int32)
        # remote rows in kv_all: partner slot is (1 - hh)
        rrow = ((1 - hh) * 128 + np.arange(128)).astype(np.int32)[:, None]
        # query bookkeeping: canonical batch order j = qb*128 + p
        g = patch_idx[b].astype(np.int64)               # [P]
        own = (g // SL) == hh
        qidx = np.where(own, g % SL, 0).astype(np.int32).reshape(2, 128).T.copy()
        qmask = own.astype(np.float32).reshape(2, 128).T.copy()
        qsel = (hh * PL + np.arange(128)).astype(np.int32)[:, None]
        m = {"table": table, "idx": idx, "qidx": qidx, "qmask": qmask,
             "qsel": qsel, "rrow": rrow}
        m.update(weights)
        in_maps.append(m)
    return in_maps


def assemble_output(results):
    out = np.zeros((B, P, D), dtype=np.float32)
    for c in range(N_CORES):
        b, hh = c // 2, c % 2
        out[b, hh * PL:(hh + 1) * PL, :] = results[c]["out"]
    return out


def kernel(**inputs):
    nc = _get_program()
    in_maps = make_in_maps(inputs)
    res = run_bass_kernel_spmd(nc, in_maps, core_ids=list(range(N_CORES)))
    return assemble_output(res.results)


if __name__ == "__main__":
    pass
